# revision 2
# baseline (speedup 1.0000x reference)
"""8-core Trainium2 attention kernel (Bass/Tile), nn_AttentionLayer.

Reference computation (B=2, S=4096, E=512, H=8, DH=64, scale=H=8):
    q = x @ Wq ; k = x @ Wk ; v = x @ Wv        (per batch)
    per head: scores = (q_h @ k_h^T) / 8 ; P = softmax(scores)
    out_h = P @ v_h ; concat heads

Sharding (no collectives needed): core = b*4 + hp handles batch b and head
pair hp (2 heads = 128 weight columns). Each core's output slice is
independent; host concatenates.

Device-side layout tricks (host does all transposes / casts / final divide):
  - host passes xT = x[b].T (bf16), per-head-pair weight slices (bf16)
  - scores computed TRANSPOSED (S^T[j,i]) so no on-device transposes anywhere
  - softmax denominator via a ones-column appended to V (stationary M=65)
  - device returns unnormalized O^T (64 rows) + denominator row per head;
    host divides and transposes back.
"""

import numpy as np
import ml_dtypes

B, S, E, H = 2, 4096, 512, 8
DH = 64
SCALE = 8.0
N_CORES = 8
EC = E // 128   # 4 e-chunks (contraction chunks for projections)
NJ = S // 128   # 32 j-chunks
NI = S // 512   # 8 i-tiles
NS = S // 512   # 8 s-tiles (q/k projections)
NSC = S // 128  # 32 s-chunks (v projection)

_NC_CACHE = None


def _emit(tc, bass, mybir):
    from contextlib import ExitStack

    f32 = mybir.dt.float32
    bf16 = mybir.dt.bfloat16
    Exp = mybir.ActivationFunctionType.Exp
    nc = tc.nc

    xT_t = nc.dram_tensor("xT", [E, S], bf16, kind="ExternalInput")
    wq_t = nc.dram_tensor("wq", [E, 128], bf16, kind="ExternalInput")
    wk_t = nc.dram_tensor("wk", [E, 128], bf16, kind="ExternalInput")
    wv_t = nc.dram_tensor("wv", [E, 128], bf16, kind="ExternalInput")
    out_t = nc.dram_tensor("out", [130, S], f32, kind="ExternalOutput")

    with ExitStack() as ctx:
        singles = ctx.enter_context(tc.tile_pool(name="singles", bufs=1))

        # ---- load inputs ----
        xt_sb = []
        for c in range(EC):
            t = singles.tile([128, S], bf16, name=f"xt{c}")
            nc.sync.dma_start(out=t, in_=xT_t[c * 128 : (c + 1) * 128, :])
            xt_sb.append(t)
        w_sb = {}
        for nm, t_dram in (("wq", wq_t), ("wk", wk_t), ("wv", wv_t)):
            t = singles.tile([128, EC, 128], bf16, name=f"{nm}sb")
            nc.sync.dma_start(
                out=t, in_=t_dram[:, :].rearrange("(c p) d -> p c d", p=128)
            )
            w_sb[nm] = t

        qT = singles.tile([128, S], bf16, name="qT")
        kT = singles.tile([128, S], bf16, name="kT")
        v_aug = singles.tile([128, NSC, 130], bf16, name="v_aug")
        nc.vector.memset(v_aug, 1.0)

        # ---- q/k projections (kT first so attention can start early) ----
        # qT/kT: [d=128(2 heads), s] = sum_e W[e,d] * xT[e,s]
        ppool = ctx.enter_context(tc.tile_pool(name="ppsum", bufs=2, space="PSUM"))
        for nm, dst in (("wk", kT), ("wq", qT)):
            for st in range(NS):
                s_sl = slice(st * 512, (st + 1) * 512)
                ps = ppool.tile([128, 512], f32, name="pp", tag="pp")
                for c in range(EC):
                    nc.tensor.matmul(
                        ps,
                        w_sb[nm][:, c, :],
                        xt_sb[c][:, s_sl],
                        start=(c == 0),
                        stop=(c == EC - 1),
                    )
                nc.vector.tensor_copy(dst[:, s_sl], ps)

        # ---- attention (v projection interleaved into i-tile 0) ----
        spool = ctx.enter_context(tc.tile_pool(name="spsum", bufs=2, space="PSUM"))
        opool = ctx.enter_context(tc.tile_pool(name="opsum", bufs=1, space="PSUM"))
        epool = ctx.enter_context(tc.tile_pool(name="epool", bufs=4))
        osb_pool = ctx.enter_context(tc.tile_pool(name="osb", bufs=2))

        for it in range(NI):
            i_sl = slice(it * 512, (it + 1) * 512)
            o_ps = [
                opool.tile([65, 512], f32, name=f"o{h}", tag=f"o{h}") for h in range(2)
            ]
            for jc in range(NJ):
                j_sl = slice(jc * 128, (jc + 1) * 128)
                if it == 0:
                    # v natural [s=128, d=128] = sum_e xT[e,s] * Wv[e,d],
                    # produced just-in-time for this j-chunk's ph2
                    ps = ppool.tile([128, 512], f32, name="pp", tag="pp")
                    pv = ps[:, 0:128]
                    for c in range(EC):
                        nc.tensor.matmul(
                            pv,
                            xt_sb[c][:, j_sl],
                            w_sb["wv"][:, c, :],
                            start=(c == 0),
                            stop=(c == EC - 1),
                        )
                    nc.vector.tensor_copy(v_aug[:, jc, 0:64], pv[:, 0:64])
                    nc.vector.tensor_copy(v_aug[:, jc, 65:129], pv[:, 64:128])
                # S^T[j,i] = sum_d kT[d,j] * qT[d,i]; both heads packed via
                # row tiling (K=64 each) into one 2-bank psum pair.
                s_pair = spool.tile([128, 1024], f32, name="s_pair", tag="s_pair")
                for h in range(2):
                    d_sl = slice(h * 64, (h + 1) * 64)
                    nc.tensor.matmul(
                        s_pair[:, h * 512 : (h + 1) * 512],
                        kT[d_sl, j_sl],
                        qT[d_sl, i_sl],
                        start=True,
                        stop=True,
                        tile_position=(h * 64, 0),
                    )
                # exp(S^T/8) for both heads in one ACT instr; cast to bf16
                e_pair = epool.tile([128, 1024], bf16, name="e_pair", tag="e_pair")
                nc.scalar.activation(e_pair, s_pair, Exp, scale=1.0 / SCALE)
                # O^T[d,i] (+ denominator row 64) accumulated over j
                for h in range(2):
                    nc.tensor.matmul(
                        o_ps[h],
                        v_aug[:, jc, h * 65 : (h + 1) * 65],
                        e_pair[:, h * 512 : (h + 1) * 512],
                        start=(jc == 0),
                        stop=(jc == NJ - 1),
                    )
            for h in range(2):
                o_sb = osb_pool.tile([65, 512], f32, name=f"osb{h}", tag=f"osb{h}")
                nc.vector.tensor_copy(o_sb, o_ps[h])
                nc.sync.dma_start(out=out_t[h * 65 : (h + 1) * 65, i_sl], in_=o_sb)


def build_nc():
    import concourse.bass as bass
    import concourse.mybir as mybir
    import concourse.tile as tile
    from concourse import bacc

    nc = bacc.Bacc(
        "TRN2", target_bir_lowering=False, debug=False, num_devices=N_CORES
    )
    with tile.TileContext(nc) as tc:
        _emit(tc, bass, mybir)
    nc.compile()
    return nc


def _get_nc():
    global _NC_CACHE
    if _NC_CACHE is None:
        _NC_CACHE = build_nc()
    return _NC_CACHE


def make_in_maps(attention_input, Wq, Wk, Wv):
    bf16 = ml_dtypes.bfloat16
    x = np.asarray(attention_input, dtype=np.float32)
    Wq = np.asarray(Wq, dtype=np.float32)
    Wk = np.asarray(Wk, dtype=np.float32)
    Wv = np.asarray(Wv, dtype=np.float32)
    in_maps = []
    for core in range(N_CORES):
        b, hp = divmod(core, 4)
        cols = slice(hp * 128, (hp + 1) * 128)
        in_maps.append(
            {
                "xT": np.ascontiguousarray(x[b].T).astype(bf16),
                "wq": np.ascontiguousarray(Wq[:, cols]).astype(bf16),
                "wk": np.ascontiguousarray(Wk[:, cols]).astype(bf16),
                "wv": np.ascontiguousarray(Wv[:, cols]).astype(bf16),
            }
        )
    return in_maps


def assemble_output(core_outs):
    """core_outs: list of 8 arrays [130, S] f32 -> full [B, S, E] f32."""
    out = np.empty((B, S, E), np.float32)
    for core in range(N_CORES):
        b, hp = divmod(core, 4)
        o = np.asarray(core_outs[core], dtype=np.float32)
        for h in range(2):
            blk = o[h * 65 : (h + 1) * 65]  # [65, S]
            onrm = blk[0:64] / blk[64:65]
            out[b, :, hp * 128 + h * 64 : hp * 128 + (h + 1) * 64] = onrm.T
    return out


def kernel(attention_input, Wq, Wk, Wv, _trace=False, **trace_kwargs):
    from concourse.bass_utils import run_bass_kernel_spmd

    nc = _get_nc()
    in_maps = make_in_maps(attention_input, Wq, Wk, Wv)
    res = run_bass_kernel_spmd(
        nc, in_maps, core_ids=list(range(N_CORES)), trace=_trace, **trace_kwargs
    )
    out = assemble_output([r["out"] for r in res.results])
    if _trace:
        kernel.last_results = res
    return out


# revision 3
# speedup vs baseline: 1.1885x; 1.1885x over previous
"""8-core Trainium2 attention kernel (Bass/Tile), nn_AttentionLayer.

Reference computation (B=2, S=4096, E=512, H=8, DH=64, scale=H=8):
    q = x @ Wq ; k = x @ Wk ; v = x @ Wv        (per batch)
    per head: scores = (q_h @ k_h^T) / 8 ; P = softmax(scores)
    out_h = P @ v_h ; concat heads

Sharding (no collectives needed): core = b*4 + hp handles batch b and head
pair hp (2 heads = 128 weight columns). Each core's output slice is
independent; host concatenates.

Device-side layout tricks (host does all transposes / casts / final divide):
  - host passes xT = x[b].T (bf16), per-head-pair weight slices (bf16)
  - scores computed TRANSPOSED (S^T[j,i]) so no on-device transposes anywhere
  - softmax denominator via a ones-column appended to V (stationary M=65)
  - device returns unnormalized O^T (64 rows) + denominator row per head;
    host divides and transposes back.
"""

import numpy as np
import ml_dtypes

B, S, E, H = 2, 4096, 512, 8
DH = 64
SCALE = 8.0
N_CORES = 8
EC = E // 128   # 4 e-chunks (contraction chunks for projections)
NJ = S // 128   # 32 j-chunks
NI = S // 512   # 8 i-tiles
NS = S // 512   # 8 s-tiles (q/k projections)
NSC = S // 128  # 32 s-chunks (v projection)

_NC_CACHE = None


def _emit(tc, bass, mybir):
    from contextlib import ExitStack

    f32 = mybir.dt.float32
    bf16 = mybir.dt.bfloat16
    Exp = mybir.ActivationFunctionType.Exp
    nc = tc.nc

    xT_t = nc.dram_tensor("xT", [E, S], bf16, kind="ExternalInput")
    wq_t = nc.dram_tensor("wq", [E, 128], bf16, kind="ExternalInput")
    wk_t = nc.dram_tensor("wk", [E, 128], bf16, kind="ExternalInput")
    wv_t = nc.dram_tensor("wv", [E, 128], bf16, kind="ExternalInput")
    out_t = nc.dram_tensor("out", [130, S], f32, kind="ExternalOutput")

    with ExitStack() as ctx:
        singles = ctx.enter_context(tc.tile_pool(name="singles", bufs=1))

        # ---- load inputs ----
        xt_sb = []
        for c in range(EC):
            t = singles.tile([128, S], bf16, name=f"xt{c}")
            nc.sync.dma_start(out=t, in_=xT_t[c * 128 : (c + 1) * 128, :])
            xt_sb.append(t)
        w_sb = {}
        for nm, t_dram in (("wq", wq_t), ("wk", wk_t), ("wv", wv_t)):
            t = singles.tile([128, EC, 128], bf16, name=f"{nm}sb")
            nc.sync.dma_start(
                out=t, in_=t_dram[:, :].rearrange("(c p) d -> p c d", p=128)
            )
            w_sb[nm] = t

        qT = singles.tile([128, S], bf16, name="qT")
        kT = singles.tile([128, S], bf16, name="kT")
        v_aug = singles.tile([128, NSC, 130], bf16, name="v_aug")
        nc.vector.memset(v_aug, 1.0)

        # ---- q/k projections (kT first so attention can start early) ----
        # qT/kT: [d=128(2 heads), s] = sum_e W[e,d] * xT[e,s]
        ppool = ctx.enter_context(tc.tile_pool(name="ppsum", bufs=2, space="PSUM"))
        for nm, dst in (("wk", kT), ("wq", qT)):
            for st in range(NS):
                s_sl = slice(st * 512, (st + 1) * 512)
                ps = ppool.tile([128, 512], f32, name="pp", tag="pp")
                for c in range(EC):
                    nc.tensor.matmul(
                        ps,
                        w_sb[nm][:, c, :],
                        xt_sb[c][:, s_sl],
                        start=(c == 0),
                        stop=(c == EC - 1),
                    )
                nc.vector.tensor_copy(dst[:, s_sl], ps)

        # ---- attention (v projection interleaved into i-tile 0) ----
        spool = ctx.enter_context(tc.tile_pool(name="spsum", bufs=2, space="PSUM"))
        opool = ctx.enter_context(tc.tile_pool(name="opsum", bufs=1, space="PSUM"))
        epool = ctx.enter_context(tc.tile_pool(name="epool", bufs=4))
        osb_pool = ctx.enter_context(tc.tile_pool(name="osb", bufs=2))

        def emit_ph1(it, jc):
            # S^T[j,i] = sum_d kT[d,j] * qT[d,i]; both heads packed via
            # row tiling (K=64 each) into one 2-bank psum pair.
            i_sl = slice(it * 512, (it + 1) * 512)
            j_sl = slice(jc * 128, (jc + 1) * 128)
            s_pair = spool.tile([128, 1024], f32, name="s_pair", tag="s_pair")
            for h in range(2):
                d_sl = slice(h * 64, (h + 1) * 64)
                nc.tensor.matmul(
                    s_pair[:, h * 512 : (h + 1) * 512],
                    kT[d_sl, j_sl],
                    qT[d_sl, i_sl],
                    start=True,
                    stop=True,
                    tile_position=(h * 64, 0),
                )
            return s_pair

        def emit_vproj(jc):
            # v natural [s=128, d=128] = sum_e xT[e,s] * Wv[e,d],
            # produced just-in-time for i-tile 0's ph2 on this j-chunk
            j_sl = slice(jc * 128, (jc + 1) * 128)
            ps = ppool.tile([128, 512], f32, name="pp", tag="pp")
            pv = ps[:, 0:128]
            for c in range(EC):
                nc.tensor.matmul(
                    pv,
                    xt_sb[c][:, j_sl],
                    w_sb["wv"][:, c, :],
                    start=(c == 0),
                    stop=(c == EC - 1),
                )
            nc.vector.tensor_copy(v_aug[:, jc, 0:64], pv[:, 0:64])
            nc.vector.tensor_copy(v_aug[:, jc, 65:129], pv[:, 64:128])

        for it in range(NI):
            i_sl = slice(it * 512, (it + 1) * 512)
            o_ps = [
                opool.tile([65, 512], f32, name=f"o{h}", tag=f"o{h}") for h in range(2)
            ]
            # software pipeline: ph1 runs one j-chunk ahead so the scalar
            # engine's exp stream never waits on the serialized
            # ACT->ph2->ph1->ACT chain.
            if it == 0:
                emit_vproj(0)
            s_cur = emit_ph1(it, 0)
            for jc in range(NJ):
                if it == 0 and jc + 1 < NJ:
                    emit_vproj(jc + 1)
                s_next = emit_ph1(it, jc + 1) if jc + 1 < NJ else None
                # exp(S^T/8) for both heads in one ACT instr; cast to bf16
                e_pair = epool.tile([128, 1024], bf16, name="e_pair", tag="e_pair")
                nc.scalar.activation(e_pair, s_cur, Exp, scale=1.0 / SCALE)
                s_cur = s_next
                # O^T[d,i] (+ denominator row 64) accumulated over j
                for h in range(2):
                    nc.tensor.matmul(
                        o_ps[h],
                        v_aug[:, jc, h * 65 : (h + 1) * 65],
                        e_pair[:, h * 512 : (h + 1) * 512],
                        start=(jc == 0),
                        stop=(jc == NJ - 1),
                    )
            for h in range(2):
                o_sb = osb_pool.tile([65, 512], f32, name=f"osb{h}", tag=f"osb{h}")
                nc.vector.tensor_copy(o_sb, o_ps[h])
                nc.sync.dma_start(out=out_t[h * 65 : (h + 1) * 65, i_sl], in_=o_sb)


def build_nc():
    import concourse.bass as bass
    import concourse.mybir as mybir
    import concourse.tile as tile
    from concourse import bacc

    nc = bacc.Bacc(
        "TRN2", target_bir_lowering=False, debug=False, num_devices=N_CORES
    )
    with tile.TileContext(nc) as tc:
        _emit(tc, bass, mybir)
    nc.compile()
    return nc


def _get_nc():
    global _NC_CACHE
    if _NC_CACHE is None:
        _NC_CACHE = build_nc()
    return _NC_CACHE


def make_in_maps(attention_input, Wq, Wk, Wv):
    bf16 = ml_dtypes.bfloat16
    x = np.asarray(attention_input, dtype=np.float32)
    Wq = np.asarray(Wq, dtype=np.float32)
    Wk = np.asarray(Wk, dtype=np.float32)
    Wv = np.asarray(Wv, dtype=np.float32)
    in_maps = []
    for core in range(N_CORES):
        b, hp = divmod(core, 4)
        cols = slice(hp * 128, (hp + 1) * 128)
        in_maps.append(
            {
                "xT": np.ascontiguousarray(x[b].T).astype(bf16),
                "wq": np.ascontiguousarray(Wq[:, cols]).astype(bf16),
                "wk": np.ascontiguousarray(Wk[:, cols]).astype(bf16),
                "wv": np.ascontiguousarray(Wv[:, cols]).astype(bf16),
            }
        )
    return in_maps


def assemble_output(core_outs):
    """core_outs: list of 8 arrays [130, S] f32 -> full [B, S, E] f32."""
    out = np.empty((B, S, E), np.float32)
    for core in range(N_CORES):
        b, hp = divmod(core, 4)
        o = np.asarray(core_outs[core], dtype=np.float32)
        for h in range(2):
            blk = o[h * 65 : (h + 1) * 65]  # [65, S]
            onrm = blk[0:64] / blk[64:65]
            out[b, :, hp * 128 + h * 64 : hp * 128 + (h + 1) * 64] = onrm.T
    return out


def kernel(attention_input, Wq, Wk, Wv, _trace=False, **trace_kwargs):
    from concourse.bass_utils import run_bass_kernel_spmd

    nc = _get_nc()
    in_maps = make_in_maps(attention_input, Wq, Wk, Wv)
    res = run_bass_kernel_spmd(
        nc, in_maps, core_ids=list(range(N_CORES)), trace=_trace, **trace_kwargs
    )
    out = assemble_output([r["out"] for r in res.results])
    if _trace:
        kernel.last_results = res
    return out


# revision 6
# speedup vs baseline: 1.1939x; 1.0045x over previous
"""8-core Trainium2 attention kernel (Bass/Tile), nn_AttentionLayer.

Reference computation (B=2, S=4096, E=512, H=8, DH=64, scale=H=8):
    q = x @ Wq ; k = x @ Wk ; v = x @ Wv        (per batch)
    per head: scores = (q_h @ k_h^T) / 8 ; P = softmax(scores)
    out_h = P @ v_h ; concat heads

Sharding (no collectives needed): core = b*4 + hp handles batch b and head
pair hp (2 heads = 128 weight columns). Each core's output slice is
independent; host concatenates.

Device-side layout tricks (host does all transposes / casts / final divide):
  - host passes xT = x[b].T (bf16), per-head-pair weight slices (bf16)
  - scores computed TRANSPOSED (S^T[j,i]) so no on-device transposes anywhere
  - softmax denominator via a ones-column appended to V (stationary M=65)
  - device returns unnormalized O^T (64 rows) + denominator row per head;
    host divides and transposes back.
"""

import numpy as np
import ml_dtypes

B, S, E, H = 2, 4096, 512, 8
DH = 64
SCALE = 8.0
N_CORES = 8
EC = E // 128   # 4 e-chunks (contraction chunks for projections)
NJ = S // 128   # 32 j-chunks
NI = S // 512   # 8 i-tiles
NS = S // 512   # 8 s-tiles (q/k projections)
NSC = S // 128  # 32 s-chunks (v projection)

_NC_CACHE = None


def _emit(tc, bass, mybir):
    from contextlib import ExitStack

    f32 = mybir.dt.float32
    bf16 = mybir.dt.bfloat16
    Exp = mybir.ActivationFunctionType.Exp
    nc = tc.nc

    xT_t = nc.dram_tensor("xT", [E, S], bf16, kind="ExternalInput")
    wq_t = nc.dram_tensor("wq", [E, 128], bf16, kind="ExternalInput")
    wk_t = nc.dram_tensor("wk", [E, 128], bf16, kind="ExternalInput")
    wv_t = nc.dram_tensor("wv", [E, 128], bf16, kind="ExternalInput")
    out_t = nc.dram_tensor("out", [130, S], f32, kind="ExternalOutput")

    with ExitStack() as ctx:
        singles = ctx.enter_context(tc.tile_pool(name="singles", bufs=1))

        # ---- load inputs (weights first so projections start early) ----
        w_sb = {}
        for nm, t_dram in (("wk", wk_t), ("wq", wq_t)):
            t = singles.tile([128, EC, 128], bf16, name=f"{nm}sb")
            nc.sync.dma_start(
                out=t, in_=t_dram[:, :].rearrange("(c p) d -> p c d", p=128)
            )
            w_sb[nm] = t
        xt_sb = []
        for c in range(EC):
            t = singles.tile([128, S], bf16, name=f"xt{c}")
            nc.sync.dma_start(out=t, in_=xT_t[c * 128 : (c + 1) * 128, :])
            xt_sb.append(t)
        for nm, t_dram in (("wv", wv_t),):
            t = singles.tile([128, EC, 128], bf16, name=f"{nm}sb")
            nc.sync.dma_start(
                out=t, in_=t_dram[:, :].rearrange("(c p) d -> p c d", p=128)
            )
            w_sb[nm] = t

        qT = singles.tile([128, S], bf16, name="qT")
        kT = singles.tile([128, S], bf16, name="kT")
        v_aug = singles.tile([128, NSC, 130], bf16, name="v_aug")
        nc.vector.memset(v_aug, 1.0)

        # ---- q/k projections ----
        # qT/kT: [d=128(2 heads), s] = sum_e W[e,d] * xT[e,s]
        ppool = ctx.enter_context(tc.tile_pool(name="ppsum", bufs=2, space="PSUM"))

        def emit_kq(nm, dst, st):
            s_sl = slice(st * 512, (st + 1) * 512)
            ps = ppool.tile([128, 512], f32, name="pp", tag="pp")
            for c in range(EC):
                nc.tensor.matmul(
                    ps,
                    w_sb[nm][:, c, :],
                    xt_sb[c][:, s_sl],
                    start=(c == 0),
                    stop=(c == EC - 1),
                )
            nc.vector.tensor_copy(dst[:, s_sl], ps)

        # prologue: full kT (attention i-tile 0 scans all of k), qT i-tile 0
        # only; remaining qT s-tiles stream inside the attention loop.
        for st in range(NS):
            emit_kq("wk", kT, st)
        emit_kq("wq", qT, 0)

        # ---- attention (v projection interleaved into i-tile 0) ----
        spool = ctx.enter_context(tc.tile_pool(name="spsum", bufs=2, space="PSUM"))
        opool = ctx.enter_context(tc.tile_pool(name="opsum", bufs=1, space="PSUM"))
        epool = ctx.enter_context(tc.tile_pool(name="epool", bufs=4))
        osb_pool = ctx.enter_context(tc.tile_pool(name="osb", bufs=2))

        def emit_ph1(it, jc):
            # S^T[j,i] = sum_d kT[d,j] * qT[d,i]; both heads packed via
            # row tiling (K=64 each) into one 2-bank psum pair.
            i_sl = slice(it * 512, (it + 1) * 512)
            j_sl = slice(jc * 128, (jc + 1) * 128)
            s_pair = spool.tile([128, 1024], f32, name="s_pair", tag="s_pair")
            for h in range(2):
                d_sl = slice(h * 64, (h + 1) * 64)
                nc.tensor.matmul(
                    s_pair[:, h * 512 : (h + 1) * 512],
                    kT[d_sl, j_sl],
                    qT[d_sl, i_sl],
                    start=True,
                    stop=True,
                    tile_position=(h * 64, 0),
                )
            return s_pair

        def emit_vproj(jc):
            # v natural [s=128, d=128] = sum_e xT[e,s] * Wv[e,d],
            # produced just-in-time for i-tile 0's ph2 on this j-chunk
            j_sl = slice(jc * 128, (jc + 1) * 128)
            ps = ppool.tile([128, 512], f32, name="pp", tag="pp")
            pv = ps[:, 0:128]
            for c in range(EC):
                nc.tensor.matmul(
                    pv,
                    xt_sb[c][:, j_sl],
                    w_sb["wv"][:, c, :],
                    start=(c == 0),
                    stop=(c == EC - 1),
                )
            nc.vector.tensor_copy(v_aug[:, jc, 0:64], pv[:, 0:64])
            nc.vector.tensor_copy(v_aug[:, jc, 65:129], pv[:, 64:128])

        # Flat software-pipelined stream over all (i-tile, j-chunk) units:
        # ph1 runs one unit ahead of the exp so the scalar engine never
        # stalls on the ACT->ph2->ph1->ACT chain, including across i-tile
        # boundaries. v projection rides i-tile 0; qT s-tiles stream in
        # mid-i-tile (needed one i-tile later).
        NU = NI * NJ
        if True:
            emit_vproj(0)
            s_cur = emit_ph1(0, 0)
            o_ps = None
            for u in range(NU):
                it, jc = divmod(u, NJ)
                i_sl = slice(it * 512, (it + 1) * 512)
                if jc == 0:
                    o_ps = [
                        opool.tile([65, 512], f32, name=f"o{h}", tag=f"o{h}")
                        for h in range(2)
                    ]
                if it == 0 and jc + 1 < NJ:
                    emit_vproj(jc + 1)
                if jc == 16 and it + 1 < NI:
                    emit_kq("wq", qT, it + 1)
                if u + 1 < NU:
                    it2, jc2 = divmod(u + 1, NJ)
                    s_next = emit_ph1(it2, jc2)
                else:
                    s_next = None
                # exp(S^T/8) for both heads in one ACT instr; cast to bf16
                e_pair = epool.tile([128, 1024], bf16, name="e_pair", tag="e_pair")
                nc.scalar.activation(e_pair, s_cur, Exp, scale=1.0 / SCALE)
                s_cur = s_next
                # O^T[d,i] (+ denominator row 64) accumulated over j
                for h in range(2):
                    nc.tensor.matmul(
                        o_ps[h],
                        v_aug[:, jc, h * 65 : (h + 1) * 65],
                        e_pair[:, h * 512 : (h + 1) * 512],
                        start=(jc == 0),
                        stop=(jc == NJ - 1),
                    )
                if jc == NJ - 1:
                    for h in range(2):
                        o_sb = osb_pool.tile(
                            [65, 512], f32, name=f"osb{h}", tag=f"osb{h}"
                        )
                        nc.vector.tensor_copy(o_sb, o_ps[h])
                        nc.sync.dma_start(
                            out=out_t[h * 65 : (h + 1) * 65, i_sl], in_=o_sb
                        )


def build_nc():
    import concourse.bass as bass
    import concourse.mybir as mybir
    import concourse.tile as tile
    from concourse import bacc

    nc = bacc.Bacc(
        "TRN2", target_bir_lowering=False, debug=False, num_devices=N_CORES
    )
    with tile.TileContext(nc) as tc:
        _emit(tc, bass, mybir)
    nc.compile()
    return nc


def _get_nc():
    global _NC_CACHE
    if _NC_CACHE is None:
        _NC_CACHE = build_nc()
    return _NC_CACHE


def make_in_maps(attention_input, Wq, Wk, Wv):
    bf16 = ml_dtypes.bfloat16
    x = np.asarray(attention_input, dtype=np.float32)
    Wq = np.asarray(Wq, dtype=np.float32)
    Wk = np.asarray(Wk, dtype=np.float32)
    Wv = np.asarray(Wv, dtype=np.float32)
    in_maps = []
    for core in range(N_CORES):
        b, hp = divmod(core, 4)
        cols = slice(hp * 128, (hp + 1) * 128)
        in_maps.append(
            {
                "xT": np.ascontiguousarray(x[b].T).astype(bf16),
                "wq": np.ascontiguousarray(Wq[:, cols]).astype(bf16),
                "wk": np.ascontiguousarray(Wk[:, cols]).astype(bf16),
                "wv": np.ascontiguousarray(Wv[:, cols]).astype(bf16),
            }
        )
    return in_maps


def assemble_output(core_outs):
    """core_outs: list of 8 arrays [130, S] f32 -> full [B, S, E] f32."""
    out = np.empty((B, S, E), np.float32)
    for core in range(N_CORES):
        b, hp = divmod(core, 4)
        o = np.asarray(core_outs[core], dtype=np.float32)
        for h in range(2):
            blk = o[h * 65 : (h + 1) * 65]  # [65, S]
            onrm = blk[0:64] / blk[64:65]
            out[b, :, hp * 128 + h * 64 : hp * 128 + (h + 1) * 64] = onrm.T
    return out


def kernel(attention_input, Wq, Wk, Wv, _trace=False, **trace_kwargs):
    from concourse.bass_utils import run_bass_kernel_spmd

    nc = _get_nc()
    in_maps = make_in_maps(attention_input, Wq, Wk, Wv)
    res = run_bass_kernel_spmd(
        nc, in_maps, core_ids=list(range(N_CORES)), trace=_trace, **trace_kwargs
    )
    out = assemble_output([r["out"] for r in res.results])
    if _trace:
        kernel.last_results = res
    return out
